# revision 17
# baseline (speedup 1.0000x reference)
"""Causal self-attention (RoPE, 16 heads, D=1024, B=2, T=2048) on 8 TRN2 NeuronCores.

Sharding: tensor-parallel over heads — 2 heads per core. Each core computes the
qkv projection for its heads (bf16 matmuls against host-pre-transposed x),
rotary embedding (bf16 on DVE), causal attention in S^T layout (keys on PSUM
partitions so P^T = exp(S^T) feeds the attn@V matmul directly as the moving
operand, with a ones-column in V producing the softmax denominators on the
tensor engine), and a partial output projection against its slice of out_w
rows. The host sums the 8 partial projections and adds out_b.

Perf notes: the PE drops to 1.2 GHz (mid pstate) after any >~1.5us gap and
takes ~4.5us to re-ramp, so the kernel (a) runs a dummy-matmul warmup stream
at t=0 while the first DMAs land, (b) stores w/x in SBUF-layout order on the
host so startup DMAs are contiguous per partition, with w's first-needed
columns in their own DMA, (c) defers each query-block's softmax-normalization
chain into the next block's loop and emits output-projection/filler matmuls
ahead of the dependent attn@V matmuls, so the in-order PE queue always has
covering work while the scalar/DVE/gpsimd chain drains.
"""

import os

import numpy as np

import concourse.mybir as mybir
import concourse.tile as tile
from concourse import bacc
from concourse.bass_utils import run_bass_kernel_spmd

F32 = mybir.dt.float32
BF16 = mybir.dt.bfloat16
EXP = mybir.ActivationFunctionType.Exp

D = 1024
H = 16
HD = 64
B = 2
T = 2048
BT = B * T            # 4096
NCORES = 8
HLOC = H // NCORES    # 2 heads per core
NDC = D // 128        # 8 contraction chunks for the qkv projection
TBW = 512             # qkv token-block width
NTBB = T // TBW       # 4 token blocks per batch
NKB = T // 128        # 16 key blocks per (b, h)
NQ = T // 512         # 4 query super-blocks per (b, h)
NF = 3 * HLOC * HD    # 384 qkv output features per core
SCALE = float(1.0 / np.sqrt(HD))
NWARM = 24            # PE warmup matmuls while startup DMAs land


def build_nc():
    nc = bacc.Bacc("TRN2", debug=False)

    # xT / w are host-pre-arranged into SBUF layout (partition-major,
    # contiguous per partition) so their DMAs need no strided descriptors.
    xT = nc.dram_tensor("xT", [128, B * NTBB * NDC * TBW], BF16, kind="ExternalInput")
    w = nc.dram_tensor("w", [128, NDC * NF], BF16, kind="ExternalInput")
    ropeP = nc.dram_tensor("ropeP", [128, T], BF16, kind="ExternalInput")
    ropeQ = nc.dram_tensor("ropeQ", [128, T], BF16, kind="ExternalInput")
    maskb = nc.dram_tensor("maskb", [128, 128], BF16, kind="ExternalInput")
    ident = nc.dram_tensor("ident", [128, 64], BF16, kind="ExternalInput")
    wo = nc.dram_tensor("wo", [128, D], BF16, kind="ExternalInput")
    y = nc.dram_tensor("y", [BT, D], BF16, kind="ExternalOutput")

    with tile.TileContext(nc) as tc:
        with (
            tc.tile_pool(name="const", bufs=1) as const,
            tc.tile_pool(name="big", bufs=1) as big,
            tc.tile_pool(name="xt", bufs=2) as xt_pool,
            tc.tile_pool(name="raw", bufs=3) as raw_pool,
            tc.tile_pool(name="gsw", bufs=1) as gsw_pool,
            tc.tile_pool(name="rtmp", bufs=2) as rtmp_pool,
            tc.tile_pool(name="p_sb", bufs=3) as p_pool,
            tc.tile_pool(name="r_sb", bufs=2) as r_pool,
            tc.tile_pool(name="rb_sb", bufs=2) as rb_pool,
            tc.tile_pool(name="aTb", bufs=3) as aT_pool,
            tc.tile_pool(name="y_sb", bufs=3) as y_pool,
            tc.tile_pool(name="aux_ps", bufs=2, space="PSUM") as aux_psum,
            tc.tile_pool(name="s_ps", bufs=2, space="PSUM") as s_psum,
            tc.tile_pool(name="o_ps", bufs=2, space="PSUM") as o_psum,
        ):
            # PE warmup: dummy matmuls with no DMA dependency keep the PE
            # busy from the end of the framework preamble so it is at full
            # clock when the first real matmul's inputs land.
            warm = const.tile([128, 512], BF16, name="warm")
            nc.gpsimd.memset(warm[:], 0.25)
            for wi in range(NWARM):
                wps = aux_psum.tile([64, 512], F32, name=f"wps{wi}", tag="aux")
                nc.tensor.matmul(
                    wps[:], warm[:, 0:64], warm[:], start=True, stop=True,
                )

            # w ft=0 columns first (all the first matmul group needs), then
            # the first x block (emitted by the first qkv thunk), then the
            # rest of w. Other constants go on the scalar engine's queue.
            w_sb = const.tile([128, NDC, NF], BF16)
            nc.sync.dma_start(
                w_sb[:, :, 0:128],
                w[:].rearrange("p (dc f) -> p dc f", dc=NDC)[:, :, 0:128])

            def load_w_rest():
                nc.sync.dma_start(
                    w_sb[:, :, 128:NF],
                    w[:].rearrange("p (dc f) -> p dc f", dc=NDC)[:, :, 128:NF])

            P_sb = const.tile([128, T], BF16)
            Q_sb = const.tile([128, T], BF16)
            mb_sb = const.tile([128, 128], BF16)
            id_sb = const.tile([128, 64], BF16)
            wo_sb = const.tile([128, D], BF16)

            def load_consts_1():
                nc.scalar.dma_start(P_sb[:], ropeP[:])
                nc.scalar.dma_start(Q_sb[:], ropeQ[:])

            def load_consts_2():
                nc.scalar.dma_start(mb_sb[:], maskb[:])
                nc.scalar.dma_start(id_sb[:], ident[:])
                nc.scalar.dma_start(wo_sb[:], wo[:])

            qrot = big.tile([128, BT], BF16, tag="qrot")
            krot = big.tile([128, BT], BF16, tag="krot")
            vsb = [
                big.tile([128, NKB, 65], BF16, name=f"v{i}", tag=f"v{i}")
                for i in range(B * HLOC)
            ]

            pending_outproj = []
            pending_chain = []
            slot = [0]  # global kb-iteration counter across both batches

            def emit_outproj_piece(job, i):
                aTb, row_base = job
                row0 = row_base + i * 128
                ysb = y_pool.tile([128, 2, 512], BF16, name=f"ysb{row0}", tag="ysb")
                for nb in range(2):
                    yps = aux_psum.tile(
                        [128, 512], F32, name=f"yps{row0}{nb}", tag="aux"
                    )
                    nc.tensor.matmul(
                        yps[:],
                        aTb[:, i * 128:(i + 1) * 128],
                        wo_sb[:, nb * 512:(nb + 1) * 512],
                        start=True,
                        stop=True,
                    )
                    nc.vector.tensor_copy(ysb[:, nb, :], yps[:])
                nc.scalar.dma_start(
                    y[row0:row0 + 128, :],
                    ysb[:].rearrange("p a t -> p (a t)"),
                )

            def pop_outproj_piece(n=1):
                for _ in range(n):
                    if not pending_outproj:
                        return
                    job, i, ready = pending_outproj[0]
                    if slot[0] < ready:
                        return
                    emit_outproj_piece(job, i)
                    if i == 3:
                        pending_outproj.pop(0)
                    else:
                        pending_outproj[0] = (job, i + 1, ready)

            def flush_chain():
                while pending_chain:
                    pending_chain.pop(0)()

            def phase1_thunks(b):
                """Emission thunks for batch b's qkv + rope + V-transpose."""
                state = {}

                def get_raws():
                    if "raws" not in state:
                        state["raws"] = [
                            raw_pool.tile(
                                [128, T], BF16, name=f"raw{b}{ft}", tag="raw"
                            )
                            for ft in range(3)
                        ]
                    return state["raws"]

                def qkv_group(tb, ft):
                    raws = get_raws()
                    if ft == 0:
                        blk = b * NTBB + tb
                        xt_t = xt_pool.tile(
                            [128, NDC, TBW], BF16, name=f"xt{b}{tb}", tag="xt"
                        )
                        nc.sync.dma_start(
                            xt_t[:],
                            xT[:, blk * NDC * TBW:(blk + 1) * NDC * TBW].rearrange(
                                "p (dc t) -> p dc t", dc=NDC
                            ),
                        )
                        state[("xt", tb)] = xt_t
                    xt_t = state[("xt", tb)]
                    ps = aux_psum.tile(
                        [128, TBW], F32, name=f"qkvps{b}{tb}{ft}", tag="aux"
                    )
                    for dc in range(NDC):
                        nc.tensor.matmul(
                            ps[:],
                            w_sb[:, dc, ft * 128:(ft + 1) * 128],
                            xt_t[:, dc, :],
                            start=(dc == 0),
                            stop=(dc == NDC - 1),
                        )
                    dst = raws[ft][:, tb * TBW:(tb + 1) * TBW]
                    if b == 0 and (tb * 3 + ft) % 2 == 0:
                        nc.scalar.copy(dst, ps[:])
                    else:
                        nc.vector.tensor_copy(dst, ps[:])

                def rope(which, tb):
                    raws = get_raws()
                    raw = raws[0] if which == 0 else raws[1]
                    rot = qrot if which == 0 else krot
                    cs = slice(tb * TBW, (tb + 1) * TBW)
                    gsw = gsw_pool.tile(
                        [128, TBW], BF16, name=f"gsw{b}{which}{tb}", tag="gsw"
                    )
                    for l in range(HLOC):
                        p0 = l * 64
                        nc.gpsimd.dma_start(
                            gsw[p0:p0 + 32, :], raw[p0 + 32:p0 + 64, cs]
                        )
                        nc.gpsimd.dma_start(
                            gsw[p0 + 32:p0 + 64, :], raw[p0:p0 + 32, cs]
                        )
                    t1 = rtmp_pool.tile(
                        [128, TBW], BF16, name=f"rt{b}{which}{tb}", tag="rt"
                    )
                    nc.vector.tensor_mul(t1[:], raw[:, cs], P_sb[:, cs])
                    nc.vector.tensor_mul(gsw[:], gsw[:], Q_sb[:, cs])
                    nc.vector.tensor_add(rot[:, b * T + tb * TBW:b * T + (tb + 1) * TBW],
                                         t1[:], gsw[:])

                def vt_group(tb, l):
                    """Transpose the 4 key blocks of token-block tb for head l."""
                    vraw = get_raws()[2]
                    bh = b * HLOC + l
                    if tb == 0 and l == 0:
                        for _l in range(HLOC):
                            nc.gpsimd.memset(vsb[b * HLOC + _l][:, :, 64], 1.0)
                    for kb in range(tb * 4, tb * 4 + 4):
                        tp = aux_psum.tile(
                            [128, 64], BF16, name=f"tp{b}{l}{kb}", tag="aux"
                        )
                        nc.tensor.transpose(
                            tp[:],
                            vraw[l * 64:(l + 1) * 64,
                                 kb * 128:(kb + 1) * 128],
                            id_sb[l * 64:(l + 1) * 64, :],
                        )
                        nc.vector.tensor_copy(vsb[bh][:, kb, 0:64], tp[:])

                thunks = []
                for tb in range(NTBB):
                    for ft in range(3):
                        thunks.append(lambda tb=tb, ft=ft: qkv_group(tb, ft))
                        if ft < 2:
                            thunks.append(lambda which=ft, tb=tb: rope(which, tb))
                    for l in range(HLOC):
                        thunks.append(lambda tb=tb, l=l: vt_group(tb, l))
                return thunks

            def attention(b, filler):
                kb_count = 0
                for qb in range(NQ):
                    q0 = qb * 512
                    nkb = (q0 + 512) // 128
                    opss = []  # allocated at kb==0, after the previous
                    # query-block's chain (the old tiles' readers) is emitted

                    def s_pair(kb, _b=b, _qb=qb, _q0=q0):
                        r_off = kb - _qb * 4
                        cm = 128 * r_off if r_off >= 0 else 0
                        k0 = kb * 128
                        ksl = slice(_b * T + k0, _b * T + k0 + 128)
                        qsl = slice(_b * T + _q0 + cm, _b * T + _q0 + 512)
                        sps = s_psum.tile(
                            [128, 2, 512], F32, name=f"sps{_b}{_qb}{kb}", tag="sps",
                        )
                        for l in range(HLOC):
                            p0 = l * 64
                            nc.tensor.matmul(
                                sps[:, l, cm:512],
                                krot[p0:p0 + 64, ksl],
                                qrot[p0:p0 + 64, qsl],
                                start=True,
                                stop=True,
                            )
                        return sps

                    spss = {0: s_pair(0)}
                    for kb in range(nkb):
                        if kb + 1 < nkb:
                            spss[kb + 1] = s_pair(kb + 1)
                        r_off = kb - qb * 4  # >= 0: diagonal-region block
                        pt = p_pool.tile([128, 2, 512], BF16, tag="pt")
                        cm = 128 * r_off if r_off >= 0 else 0
                        sps = spss[kb]
                        if cm == 0:
                            nc.scalar.activation(
                                pt[:].rearrange("p a t -> p (a t)"),
                                sps[:].rearrange("p a t -> p (a t)"),
                                EXP, scale=SCALE,
                            )
                        else:
                            nc.scalar.activation(
                                pt[:, :, cm:512], sps[:, :, cm:512],
                                EXP, scale=SCALE,
                            )
                        if r_off >= 0:
                            nc.vector.tensor_mul(
                                pt[:, :, cm:cm + 128],
                                pt[:, :, cm:cm + 128],
                                mb_sb[:, None, :].broadcast_to([128, 2, 128]),
                            )
                        # Previous block's normalization chain + deferred
                        # PE work goes in front of the dependent attn@V
                        # matmuls so the PE queue stays covered.
                        flush_chain()
                        pop_outproj_piece(2 if slot[0] > 84 else 1)
                        if kb_count % 2 == 1 or kb_count >= 30:
                            f = next(filler, None)
                            if f is not None:
                                f()
                        if kb == 0:
                            opss.extend(
                                o_psum.tile(
                                    [65, 512], F32, name=f"ops{b}{qb}{_l}",
                                    tag="ops",
                                )
                                for _l in range(HLOC)
                            )
                        for l in range(HLOC):
                            nc.tensor.matmul(
                                opss[l][:, cm:512],
                                vsb[b * HLOC + l][:, kb, :],
                                pt[:, l, cm:512],
                                start=(kb == 0),
                                stop=(kb == nkb - 1),
                            )
                        del spss[kb]
                        kb_count += 1
                        slot[0] += 1

                    def chain(_opss=opss, _b=b, _q0=q0):
                        aTb = aT_pool.tile(
                            [128, 512], BF16, name=f"aTb{_b}{_q0}", tag="aTb"
                        )
                        for l in range(HLOC):
                            ops = _opss[l]
                            d_sb = r_pool.tile([1, 512], F32, tag="d")
                            nc.scalar.copy(d_sb[:], ops[64:65, :])
                            r_sb = r_pool.tile([1, 512], F32, tag="r")
                            nc.vector.reciprocal_approx_fast(r_sb[:], d_sb[:])
                            rb_sb = rb_pool.tile([64, 512], F32, tag="rb")
                            nc.gpsimd.partition_broadcast(rb_sb[:], r_sb[:])
                            nc.vector.tensor_mul(
                                aTb[l * 64:(l + 1) * 64, :], ops[0:64, :], rb_sb[:]
                            )
                        pending_outproj.append(((aTb, _b * T + _q0), 0, slot[0] + 4))

                    pending_chain.append(chain)

            th0 = phase1_thunks(0)
            th0[0]()           # qkv(0,0): xt0 DMA + first matmuls
            load_w_rest()      # w ft=1/2 columns right behind xt0
            load_consts_1()    # rope tables (needed by thunk 1)
            th0[1]()
            th0[2]()
            load_consts_2()    # mask / ident / out weights
            for th in th0[3:]:
                th()
            p1b1 = iter(phase1_thunks(1))
            attention(0, p1b1)
            for th in p1b1:
                th()
            attention(1, iter(()))

            flush_chain()
            while pending_outproj:
                job, i, _ready = pending_outproj.pop(0)
                for j in range(i, 4):
                    emit_outproj_piece(job, j)

    nc.finalize()
    return nc


def _rope_tables():
    inv_freq = 1.0 / (10000.0 ** (np.arange(0, HD, 2, dtype=np.float32) / HD))
    t = np.arange(T, dtype=np.float32)
    freqs = t[:, None] * inv_freq[None, :]                          # [T, 32]
    rope = np.concatenate([np.sin(freqs), np.cos(freqs)], axis=-1)  # [T, 64]
    sin = rope[:, ::2]    # [T, 32]  (reference's "sin")
    cos = rope[:, 1::2]   # [T, 32]  (reference's "cos")
    # rot = raw * P + swap_halves(raw) * Q  with raw rows [x1(32) ; x2(32)]:
    #  rows 0..31  (out half0 = x1*cos - x2*sin; raw=x1, swap=x2): P=cos, Q=-sin
    #  rows 32..63 (out half1 = x1*sin + x2*cos; raw=x2, swap=x1): P=cos, Q=sin
    P64 = np.concatenate([cos.T, cos.T], axis=0)                    # [64, T]
    Q64 = np.concatenate([-sin.T, sin.T], axis=0)                   # [64, T]
    P128 = np.concatenate([P64, P64], axis=0).astype(np.float32)
    Q128 = np.concatenate([Q64, Q64], axis=0).astype(np.float32)
    return np.ascontiguousarray(P128), np.ascontiguousarray(Q128)


def make_core_inputs(x, qkv_w, qkv_b, out_w):
    """Build the per-core input maps for the 8-way head-parallel kernel."""
    import ml_dtypes
    bf16 = ml_dtypes.bfloat16

    x = np.asarray(x, dtype=np.float32)
    qkv_w = np.asarray(qkv_w, dtype=np.float32)
    qkv_b = np.asarray(qkv_b, dtype=np.float32)
    out_w = np.asarray(out_w, dtype=np.float32)
    if np.max(np.abs(qkv_b)) != 0.0:
        raise NotImplementedError("kernel assumes qkv_b == 0 (spec fill: zeros)")

    # SBUF-layout x: [partition, (b tb) dc t] contiguous per partition.
    xP = np.ascontiguousarray(
        x.reshape(B * NTBB, TBW, NDC, 128).transpose(3, 0, 2, 1)
        .reshape(128, B * NTBB * NDC * TBW).astype(bf16))
    ropeP, ropeQ = _rope_tables()
    ropeP = ropeP.astype(bf16)
    ropeQ = ropeQ.astype(bf16)
    deint = np.concatenate([np.arange(0, HD, 2), np.arange(1, HD, 2)])  # [64]
    maskb = np.triu(np.ones((128, 128), dtype=np.float32)).astype(bf16)
    ident = np.concatenate([np.eye(64, dtype=np.float32)] * 2, axis=0).astype(bf16)

    in_maps = []
    for c in range(NCORES):
        cols = []
        for sect, perm in ((0, deint), (1, deint), (2, np.arange(HD))):
            for l in range(HLOC):
                g = HLOC * c + l
                cols.append(sect * D + g * HD + perm)
        cols = np.concatenate(cols)
        # SBUF-layout w: [partition, dc f] contiguous per partition.
        w_core = np.ascontiguousarray(
            qkv_w[:, cols].reshape(NDC, 128, NF).transpose(1, 0, 2)
            .reshape(128, NDC * NF).astype(bf16))
        wo_core = np.ascontiguousarray(out_w[c * 128:(c + 1) * 128, :].astype(bf16))
        in_maps.append({
            "xT": xP,
            "w": w_core,
            "ropeP": ropeP,
            "ropeQ": ropeQ,
            "maskb": maskb,
            "ident": ident,
            "wo": wo_core,
        })
    return in_maps


_NC_CACHE = None


def kernel(x, qkv_w, qkv_b, out_w, out_b):
    global _NC_CACHE
    if _NC_CACHE is None:
        _NC_CACHE = build_nc()
    nc = _NC_CACHE
    in_maps = make_core_inputs(x, qkv_w, qkv_b, out_w)
    trace = bool(os.environ.get("ATTN_KERNEL_TRACE"))
    res = run_bass_kernel_spmd(
        nc, in_maps, core_ids=list(range(NCORES)), trace=trace,
    )
    kernel.last_results = res
    y = res.results[0]["y"].astype(np.float64)
    for c in range(1, NCORES):
        y = y + res.results[c]["y"].astype(np.float64)
    y = y + np.asarray(out_b, dtype=np.float64)[None, :]
    return np.ascontiguousarray(y.reshape(B, T, D).astype(np.float32))
